# revision 1
# baseline (speedup 1.0000x reference)
# Correlation2D (RAFT-style correlation pyramid lookup) on 8 TRN2 NeuronCores.
#
# Sharding: data-parallel over the bs*h*w query axis. Each core owns 1024
# queries (= 8 image rows): it computes its slice of the cost volume via
# GEMM (fmap2 replicated, pooling folded into fmap2), writes the 4-level
# pyramid per-query-contiguous to DRAM, gathers 10x10 patches around each
# query's coords with one indirect DMA, and does the separable bilinear
# combine on-chip. Output per core is [324, 8, 128] (channel-major), host
# concatenates along y.
import numpy as np

# ---- problem constants (hardcoded per contest contract) ----
H, W = 64, 128
D = 256
NUM_LEVELS = 4
RADIUS = 4
KK = 2 * RADIUS + 1        # 9
PS = KK + 1                # 10x10 patch per (query, level)
NCORES = 8
QPC = (H * W) // NCORES    # 1024 queries per core
NBLK = QPC // 128          # 8 blocks of 128 queries
LVL_W = [W >> l for l in range(NUM_LEVELS)]            # 128 64 32 16
LVL_H = [H >> l for l in range(NUM_LEVELS)]            # 64 32 16 8
LVL_N = [LVL_W[l] * LVL_H[l] for l in range(NUM_LEVELS)]   # 8192 2048 512 128
LVL_OFF = [sum(LVL_N[:l]) for l in range(NUM_LEVELS)]  # 0 8192 10240 10752
LVLSUM = sum(LVL_N)        # 10880
PAD = 1024                 # zeroed head/tail pad (elements) of the cv buffer
QS = QPC * LVLSUM
NCH = NUM_LEVELS * KK * KK  # 324 output channels
MM_N = 512                 # matmul N-chunk (one PSUM bank of f32)

_CACHE = {}


def _emit(ctx, tc, out_ext, f1c, f2, crd, dbg=None):
    import concourse.bass as bass
    import concourse.mybir as mybir
    from concourse.masks import make_identity

    nc = tc.nc
    f32 = mybir.dt.float32
    i32 = mybir.dt.int32
    Alu = mybir.AluOpType

    const_pool = ctx.enter_context(tc.tile_pool(name="constp", bufs=1))
    f2_pool = ctx.enter_context(tc.tile_pool(name="f2p", bufs=1))
    coordp = ctx.enter_context(tc.tile_pool(name="coordp", bufs=1))
    small = ctx.enter_context(tc.tile_pool(name="small", bufs=2))
    lhsp = ctx.enter_context(tc.tile_pool(name="lhsp", bufs=2))
    cvp = ctx.enter_context(tc.tile_pool(name="cvp", bufs=1))
    big = ctx.enter_context(tc.tile_pool(name="big", bufs=1))
    outp = ctx.enter_context(tc.tile_pool(name="outp", bufs=2))
    psum = ctx.enter_context(tc.tile_pool(name="psum", bufs=6, space="PSUM"))
    psum_t = ctx.enter_context(tc.tile_pool(name="psumt", bufs=2, space="PSUM"))
    dramp = ctx.enter_context(tc.tile_pool(name="dramp", bufs=1, space="DRAM"))

    # ---------------- DRAM cv buffer (per-query contiguous pyramid) --------
    cv_dram = dramp.tile([PAD + QS + PAD], f32, name="cv_dram")

    # zero the head/tail pads (garbage there is gathered but must stay finite;
    # it is multiplied by a zero weight)
    ztile = const_pool.tile([128, 8], f32, name="ztile")
    nc.vector.memset(ztile[:], 0.0)
    nc.scalar.dma_start(
        out=cv_dram[0:PAD].rearrange("(p b) -> p b", p=128), in_=ztile[:]
    )
    nc.scalar.dma_start(
        out=cv_dram[PAD + QS : PAD + QS + PAD].rearrange("(p b) -> p b", p=128),
        in_=ztile[:],
    )

    # ---------------- coords -> indices / weights --------------------------
    # cx/cy for this core's queries: [128, 8]  (partition p = x, free b = y row)
    cxs = coordp.tile([128, NBLK], f32, name="cxs")
    cys = coordp.tile([128, NBLK], f32, name="cys")
    nc.scalar.dma_start(out=cxs[:], in_=crd[0, :].rearrange("(b p) -> p b", p=128))
    nc.scalar.dma_start(out=cys[:], in_=crd[1, :].rearrange("(b p) -> p b", p=128))

    # per-query element base offset of its pyramid in cv_dram
    # (iota steps are int16-limited, so build q_local then scale in f32)
    bq_i = coordp.tile([128, NBLK], i32, name="bq_i")
    nc.gpsimd.iota(bq_i[:], pattern=[[128, NBLK]], base=0, channel_multiplier=1)
    bqf = coordp.tile([128, NBLK], f32, name="bqf")
    nc.vector.tensor_copy(out=bqf[:], in_=bq_i[:])
    nc.vector.tensor_scalar(
        bqf[:], bqf[:], float(LVLSUM), float(PAD),
        op0=mybir.AluOpType.mult, op1=mybir.AluOpType.add,
    )

    idx_i = coordp.tile([128, NBLK, NUM_LEVELS, PS], i32, name="idx_i")
    wx0e = coordp.tile([128, NBLK, NUM_LEVELS, KK], f32, name="wx0e")
    wx1e = coordp.tile([128, NBLK, NUM_LEVELS, KK], f32, name="wx1e")
    wy0e = coordp.tile([128, NBLK, NUM_LEVELS, KK], f32, name="wy0e")
    wy1e = coordp.tile([128, NBLK, NUM_LEVELS, KK], f32, name="wy1e")

    # c ramp: -4..5 (patch-col -> absolute offset from floor(coord))
    cramp_i = const_pool.tile([128, PS], i32, name="cramp_i")
    nc.gpsimd.iota(cramp_i[:], pattern=[[1, PS]], base=-RADIUS, channel_multiplier=0)
    crampf = const_pool.tile([128, PS], f32, name="crampf")
    nc.vector.tensor_copy(out=crampf[:], in_=cramp_i[:])

    def floor_frac(src, lvl, nm):
        """src [128,8] f32 coords -> (floor f32, frac f32) at level lvl."""
        inv = 1.0 / (1 << lvl)
        xs = small.tile([128, NBLK], f32, name=f"xs_{nm}", tag="xs")
        nc.vector.tensor_scalar_mul(xs[:], src[:], inv)
        ii = small.tile([128, NBLK], i32, name=f"ii_{nm}", tag="ii")
        nc.vector.tensor_copy(out=ii[:], in_=xs[:])          # f32 -> i32 cast
        ff = small.tile([128, NBLK], f32, name=f"ff_{nm}", tag="ff")
        nc.vector.tensor_copy(out=ff[:], in_=ii[:])          # back to f32
        adj = small.tile([128, NBLK], f32, name=f"adj_{nm}", tag="adj")
        nc.vector.tensor_tensor(adj[:], ff[:], xs[:], op=Alu.is_gt)
        nc.vector.tensor_tensor(ff[:], ff[:], adj[:], op=Alu.subtract)  # floor
        fr = small.tile([128, NBLK], f32, name=f"fr_{nm}", tag="fr")
        nc.vector.tensor_tensor(fr[:], xs[:], ff[:], op=Alu.subtract)   # frac
        return ff, fr

    for l in range(NUM_LEVELS):
        Wl, Hl = LVL_W[l], LVL_H[l]
        ixf, fx = floor_frac(cxs, l, f"x{l}")
        iyf, fy = floor_frac(cys, l, f"y{l}")

        # interp weights with the OOB zero-mask folded in
        for (name_t, frac, posf, lim) in (
            ((wx0e, wx1e), fx, ixf, Wl - 1),
            ((wy0e, wy1e), fy, iyf, Hl - 1),
        ):
            w0t, w1t = name_t
            # tap positions posf-4+c for c in 0..9
            pos = small.tile([128, NBLK, PS], f32, name=f"pos{l}", tag="pos")
            nc.vector.tensor_tensor(
                pos[:],
                posf[:].unsqueeze(2).to_broadcast([128, NBLK, PS]),
                crampf[:].unsqueeze(1).to_broadcast([128, NBLK, PS]),
                op=Alu.add,
            )
            ok = small.tile([128, NBLK, PS], f32, name=f"ok{l}", tag="ok")
            nc.vector.tensor_scalar(pos[:], pos[:], float(lim) / 2.0, 2.0,
                                    op0=Alu.subtract, op1=Alu.mult)
            # now pos = 2*p - lim; in-bounds <=> |pos| <= lim
            nc.scalar.activation(ok[:], pos[:], mybir.ActivationFunctionType.Abs)
            nc.vector.tensor_scalar(ok[:], ok[:], float(lim), None, op0=Alu.is_le)
            w0 = small.tile([128, NBLK], f32, name=f"w0_{l}", tag="w0")
            nc.vector.tensor_scalar(w0[:], frac[:], -1.0, 1.0,
                                    op0=Alu.mult, op1=Alu.add)  # 1 - frac
            nc.vector.tensor_tensor(
                w0t[:, :, l, :],
                w0[:].unsqueeze(2).to_broadcast([128, NBLK, KK]),
                ok[:, :, 0:KK],
                op=Alu.mult,
            )
            nc.vector.tensor_tensor(
                w1t[:, :, l, :],
                frac[:].unsqueeze(2).to_broadcast([128, NBLK, KK]),
                ok[:, :, 1:PS],
                op=Alu.mult,
            )

        # gather start indices: bq + lvl_off + (iy-4+r)*Wl + (ix-4)
        t1 = small.tile([128, NBLK], f32, name=f"t1_{l}", tag="t1")
        nc.vector.tensor_scalar_mul(t1[:], iyf[:], float(Wl))
        nc.vector.tensor_tensor(t1[:], t1[:], ixf[:], op=Alu.add)
        nc.vector.tensor_tensor(t1[:], t1[:], bqf[:], op=Alu.add)
        nc.vector.tensor_scalar_add(
            t1[:], t1[:], float(LVL_OFF[l] - RADIUS * Wl - RADIUS)
        )
        rr_i = small.tile([128, PS], i32, name=f"rri_{l}", tag="rri")
        nc.gpsimd.iota(rr_i[:], pattern=[[Wl, PS]], base=0, channel_multiplier=0)
        rrf = small.tile([128, PS], f32, name=f"rrf_{l}", tag="rrf")
        nc.vector.tensor_copy(out=rrf[:], in_=rr_i[:])
        idxf = small.tile([128, NBLK, PS], f32, name=f"idxf_{l}", tag="idxf")
        nc.vector.tensor_tensor(
            idxf[:],
            t1[:].unsqueeze(2).to_broadcast([128, NBLK, PS]),
            rrf[:].unsqueeze(1).to_broadcast([128, NBLK, PS]),
            op=Alu.add,
        )
        nc.vector.tensor_copy(out=idx_i[:, :, l, :], in_=idxf[:])  # exact ints

    # ---------------- fmap2 load + pyramid pooling -------------------------
    # f2 as two K-halves [128 chan, 8192 pix]; pooled levels keep raw SUMS,
    # the 1/16 * 0.25^l scale is folded into the PSUM drain.
    f2_lv = []
    halves = []
    for k in range(2):
        f2h = f2_pool.tile([128, LVL_N[0]], f32, name=f"f2h{k}")
        nc.sync.dma_start(out=f2h[:], in_=f2[k * 128 : (k + 1) * 128, :])
        halves.append(f2h)
    f2_lv.append(halves)
    for l in range(1, NUM_LEVELS):
        Wl, Hl = LVL_W[l], LVL_H[l]
        pw, ph = LVL_W[l - 1], LVL_H[l - 1]
        halves = []
        for k in range(2):
            prev = f2_lv[l - 1][k][:].rearrange(
                "p (h w two) -> p h w two", h=ph, w=pw // 2, two=2
            )
            s1 = small.tile(
                [128, ph, pw // 2], f32, name=f"s1_{l}_{k}", tag="poolt", bufs=1
            )
            nc.vector.tensor_tensor(
                s1[:], prev[:, :, :, 0], prev[:, :, :, 1], op=Alu.add
            )
            s1v = s1[:].rearrange("p (h2 two) w -> p h2 two w", h2=Hl, two=2)
            cur = f2_pool.tile([128, Hl * Wl], f32, name=f"f2l{l}_{k}")
            curv = cur[:].rearrange("p (h w) -> p h w", h=Hl, w=Wl)
            nc.vector.tensor_tensor(
                curv[:], s1v[:, :, 0, :], s1v[:, :, 1, :], op=Alu.add
            )
            halves.append(cur)
        f2_lv.append(halves)

    # ---------------- GEMM: cv blocks ----------------------------------
    cvq = cv_dram[PAD : PAD + QS].rearrange("(q s) -> q s", s=LVLSUM)
    drain_parity = 0
    for b in range(NBLK):
        lhs = []
        for k in range(2):
            lt = lhsp.tile([128, 128], f32, name=f"lhsT{k}", tag=f"lhsT{k}")
            nc.scalar.dma_start(
                out=lt[:], in_=f1c[k * 128 : (k + 1) * 128, b * 128 : (b + 1) * 128]
            )
            lhs.append(lt)
        for l in range(NUM_LEVELS):
            scale_l = (1.0 / 16.0) * (0.25 ** l)
            Nl = LVL_N[l]
            cv_sb = cvp.tile([128, Nl], f32, name="cv_sb", tag="cv_sb")
            for n0 in range(0, Nl, MM_N):
                n1 = min(Nl, n0 + MM_N)
                pt = psum.tile([128, n1 - n0], f32, name="pt", tag="pt")
                nc.tensor.matmul(
                    pt[:], lhs[0][:], f2_lv[l][0][:, n0:n1], start=True, stop=False
                )
                nc.tensor.matmul(
                    pt[:], lhs[1][:], f2_lv[l][1][:, n0:n1], start=False, stop=True
                )
                dst = cv_sb[:, n0:n1]
                if drain_parity % 2 == 0:
                    nc.vector.tensor_scalar_mul(dst[:], pt[:], scale_l)
                else:
                    nc.scalar.mul(dst[:], pt[:], scale_l)
                drain_parity += 1
            nc.sync.dma_start(
                out=cvq[b * 128 : (b + 1) * 128, LVL_OFF[l] : LVL_OFF[l] + Nl],
                in_=cv_sb[:],
            )

    # ---------------- gather all patches -----------------------------------
    patch = big.tile([128, NBLK, NUM_LEVELS, PS, PS], f32, name="patch")
    # HW contract for indirect DMA: ONE offset per dest partition row, each
    # covering the row's full contiguous run. So issue one gather per
    # (block, level, patch-row): dest [128, 10], offsets [128, 1].
    # in_ viewed 2-D with the indexed axis last => coef = 1, indices are raw
    # element offsets into the flat buffer.
    NTOT = PAD + QS + PAD
    cv2d = cv_dram[:].rearrange("(a b) -> a b", b=NTOT // 1024)
    for b in range(NBLK):
        for l in range(NUM_LEVELS):
            for r in range(PS):
                nc.gpsimd.indirect_dma_start(
                    out=patch[:, b, l, r, :],
                    out_offset=None,
                    in_=cv2d,
                    in_offset=bass.IndirectOffsetOnAxis(
                        ap=idx_i[:, b, l, r].unsqueeze(1), axis=1
                    ),
                )

    if dbg is not None:
        nc.sync.dma_start(
            out=dbg["idx"][:], in_=idx_i[:].rearrange("p b l r -> p (b l r)")
        )
        nc.sync.dma_start(
            out=dbg["patch"][:], in_=patch[:].rearrange("p b l r c -> p (b l r c)")
        )
        nc.sync.dma_start(
            out=dbg["wx0"][:], in_=wx0e[:].rearrange("p b l k -> p (b l k)")
        )
        nc.sync.dma_start(
            out=dbg["wy0"][:], in_=wy0e[:].rearrange("p b l k -> p (b l k)")
        )
        nc.sync.dma_start(
            out=dbg["cv"][:],
            in_=cv_dram[0 : PAD + 2 * LVLSUM].rearrange("(p x) -> p x", p=128),
        )

    # ---------------- separable bilinear -----------------------------------
    tx = big.tile([128, NBLK, NUM_LEVELS, PS, KK], f32, name="tx")
    tx2 = big.tile([128, NBLK, NUM_LEVELS, PS, KK], f32, name="tx2")
    bshape_x = [128, NBLK, NUM_LEVELS, PS, KK]
    nc.vector.tensor_tensor(
        tx[:], patch[:, :, :, :, 0:KK],
        wx0e[:].unsqueeze(3).to_broadcast(bshape_x), op=Alu.mult,
    )
    nc.vector.tensor_tensor(
        tx2[:], patch[:, :, :, :, 1:PS],
        wx1e[:].unsqueeze(3).to_broadcast(bshape_x), op=Alu.mult,
    )
    nc.vector.tensor_tensor(tx[:], tx[:], tx2[:], op=Alu.add)

    # reuse dead slots: patch is dead after tx/tx2, tx2 dead after the add
    outq = big.tile([128, NBLK, NUM_LEVELS, KK, KK], f32, name="outq", tag="patch")
    outq2 = big.tile([128, NBLK, NUM_LEVELS, KK, KK], f32, name="outq2", tag="tx2")
    bshape_y = [128, NBLK, NUM_LEVELS, KK, KK]
    nc.vector.tensor_tensor(
        outq[:], tx[:, :, :, 0:KK, :],
        wy0e[:].unsqueeze(4).to_broadcast(bshape_y), op=Alu.mult,
    )
    nc.vector.tensor_tensor(
        outq2[:], tx[:, :, :, 1:PS, :],
        wy1e[:].unsqueeze(4).to_broadcast(bshape_y), op=Alu.mult,
    )
    nc.vector.tensor_tensor(outq[:], outq[:], outq2[:], op=Alu.add)

    # ---------------- transpose to channel-major + store -------------------
    ident = const_pool.tile([128, 128], f32, name="ident")
    make_identity(nc, ident[:])
    outq_v = outq[:].rearrange("p b l dy dx -> p b (l dy dx)")
    CHUNKS = [(0, 128), (128, 128), (256, NCH - 256)]
    for k, (c0, nk) in enumerate(CHUNKS):
        outTk = outp.tile([128, NBLK, 128], f32, name="outTk", tag="outTk")
        for b in range(NBLK):
            ptt = psum_t.tile([128, 128], f32, name="ptt", tag="ptt")
            nc.tensor.transpose(
                out=ptt[:nk, :], in_=outq_v[:, b, c0 : c0 + nk], identity=ident[:]
            )
            if b % 2 == 0:
                nc.vector.tensor_copy(out=outTk[0:nk, b, :], in_=ptt[:nk, :])
            else:
                nc.scalar.copy(out=outTk[0:nk, b, :], in_=ptt[:nk, :])
        nc.sync.dma_start(
            out=out_ext[c0 : c0 + nk, :, :], in_=outTk[0:nk, :, :]
        )


def build_program(debug=False):
    """Build (once) the single-core SPMD bass program."""
    key = ("nc", debug)
    if key in _CACHE:
        return _CACHE[key]
    import concourse.tile as tile
    import concourse.mybir as mybir
    from concourse import bacc

    f32 = mybir.dt.float32
    i32 = mybir.dt.int32
    nc = bacc.Bacc(
        "TRN2",
        target_bir_lowering=False,
        debug=False,
        enable_asserts=True,
        num_devices=NCORES,
    )
    f1c = nc.dram_tensor("f1c", [D, QPC], f32, kind="ExternalInput").ap()
    f2 = nc.dram_tensor("f2", [D, H * W], f32, kind="ExternalInput").ap()
    crd = nc.dram_tensor("crd", [2, QPC], f32, kind="ExternalInput").ap()
    out = nc.dram_tensor("out", [NCH, H // NCORES, W], f32, kind="ExternalOutput").ap()
    dbg = None
    if debug:
        dbg = {
            "idx": nc.dram_tensor(
                "dbg_idx", [128, NBLK * NUM_LEVELS * PS], i32, kind="ExternalOutput"
            ).ap(),
            "patch": nc.dram_tensor(
                "dbg_patch", [128, NBLK * NUM_LEVELS * PS * PS], f32,
                kind="ExternalOutput",
            ).ap(),
            "wx0": nc.dram_tensor(
                "dbg_wx0", [128, NBLK * NUM_LEVELS * KK], f32, kind="ExternalOutput"
            ).ap(),
            "wy0": nc.dram_tensor(
                "dbg_wy0", [128, NBLK * NUM_LEVELS * KK], f32, kind="ExternalOutput"
            ).ap(),
            "cv": nc.dram_tensor(
                "dbg_cv", [128, (PAD + 2 * LVLSUM) // 128], f32, kind="ExternalOutput"
            ).ap(),
        }

    from contextlib import ExitStack

    with tile.TileContext(nc) as tc, ExitStack() as ctx:
        _emit(ctx, tc, out, f1c, f2, crd, dbg=dbg)
    nc.compile()
    _CACHE[key] = nc
    return nc


def make_in_maps(fmap1, fmap2, coords):
    f1 = np.ascontiguousarray(np.asarray(fmap1, dtype=np.float32).reshape(D, H * W))
    f2 = np.ascontiguousarray(np.asarray(fmap2, dtype=np.float32).reshape(D, H * W))
    crd = np.asarray(coords, dtype=np.float32).reshape(2, H * W)
    in_maps = []
    for c in range(NCORES):
        sl = slice(c * QPC, (c + 1) * QPC)
        in_maps.append(
            {
                "f1c": np.ascontiguousarray(f1[:, sl]),
                "f2": f2,
                "crd": np.ascontiguousarray(crd[:, sl]),
            }
        )
    return in_maps


def kernel(fmap1, fmap2, coords):
    from concourse.bass_utils import run_bass_kernel_spmd

    nc = build_program()
    in_maps = make_in_maps(fmap1, fmap2, coords)
    res = run_bass_kernel_spmd(nc, in_maps, list(range(NCORES)))
    parts = [res.results[c]["out"] for c in range(NCORES)]  # [324, 8, 128] each
    full = np.concatenate(parts, axis=1)  # [324, 64, 128]
    return full[None].astype(np.float32)



# revision 7
# speedup vs baseline: 5.0964x; 5.0964x over previous
# Correlation2D (RAFT-style correlation pyramid lookup) on 8 TRN2 NeuronCores.
#
# Sharding: data-parallel over the bs*h*w query axis. Each core owns 1024
# queries (= 8 image rows). Per block of 128 queries it computes its slice of
# the cost volume via a bf16 GEMM (fmap2 replicated, pooling folded into
# fmap2), writes the 4-level pyramid per-query-contiguous to DRAM (bf16), and
# gathers ONE contiguous run per (block, level) spanning the whole 10x10
# patch (rows are Wl apart inside the run; the bilinear stage reads the run
# through a strided view). Bilinear combine is separable in bf16; output is
# PE-transposed to channel-major. Output per core is [324, 8, 128] f32, host
# concatenates along y.
import numpy as np

# ---- problem constants (hardcoded per contest contract) ----
H, W = 64, 128
D = 256
NUM_LEVELS = 4
RADIUS = 4
KK = 2 * RADIUS + 1        # 9
PS = KK + 1                # 10x10 patch per (query, level)
NCORES = 8
QPC = (H * W) // NCORES    # 1024 queries per core
NBLK = QPC // 128          # 8 blocks of 128 queries
LVL_W = [W >> l for l in range(NUM_LEVELS)]            # 128 64 32 16
LVL_H = [H >> l for l in range(NUM_LEVELS)]            # 64 32 16 8
LVL_N = [LVL_W[l] * LVL_H[l] for l in range(NUM_LEVELS)]   # 8192 2048 512 128
LVL_OFF = [sum(LVL_N[:l]) for l in range(NUM_LEVELS)]  # 0 8192 10240 10752
LVLSUM = sum(LVL_N)        # 10880
RUN = [KK * LVL_W[l] + PS for l in range(NUM_LEVELS)]  # 1162 586 298 154
ROWL = [PS * LVL_W[l] for l in range(NUM_LEVELS)]      # 1280 640 320 160
PAD = 1024                 # zeroed head/tail pad (elements) per block buffer
BQS = 128 * LVLSUM         # elements of cv per block
BTOT = PAD + BQS + PAD     # per-block DRAM tensor elements (bf16)
NCH = NUM_LEVELS * KK * KK  # 324 output channels
MM_N = 512                 # matmul N-chunk (one PSUM bank of f32)

_CACHE = {}


def _emit(ctx, tc, out_ext, f1c, f2, crd, dbg=None):
    import concourse.bass as bass
    import concourse.mybir as mybir
    from concourse.masks import make_identity

    nc = tc.nc
    f32 = mybir.dt.float32
    bf16 = mybir.dt.bfloat16
    i32 = mybir.dt.int32
    Alu = mybir.AluOpType

    const_pool = ctx.enter_context(tc.tile_pool(name="constp", bufs=1))
    f2_pool = ctx.enter_context(tc.tile_pool(name="f2p", bufs=1))
    f1_pool = ctx.enter_context(tc.tile_pool(name="f1p", bufs=1))
    coordp = ctx.enter_context(tc.tile_pool(name="coordp", bufs=1))
    small = ctx.enter_context(tc.tile_pool(name="small", bufs=2))
    cvp = ctx.enter_context(tc.tile_pool(name="cvp", bufs=2))
    patchp = ctx.enter_context(tc.tile_pool(name="patchp", bufs=1))
    txp = ctx.enter_context(tc.tile_pool(name="txp", bufs=2))
    outp = ctx.enter_context(tc.tile_pool(name="outp", bufs=2))
    psum = ctx.enter_context(tc.tile_pool(name="psum", bufs=6, space="PSUM"))
    psum_t = ctx.enter_context(tc.tile_pool(name="psumt", bufs=2, space="PSUM"))
    dramp = ctx.enter_context(tc.tile_pool(name="dramp", bufs=1, space="DRAM"))

    # ------------- per-block DRAM cv buffers (query-contiguous pyramid) ----
    cv_dram = [dramp.tile([BTOT], bf16, name=f"cv_dram{b}") for b in range(NBLK)]

    # zero head/tail pads (gather runs can poke into them; must stay finite)
    ztile = const_pool.tile([128, 8], bf16, name="ztile")
    nc.vector.memset(ztile[:], 0.0)
    for b in range(NBLK):
        nc.sync.dma_start(
            out=cv_dram[b][0:PAD].rearrange("(p x) -> p x", p=128), in_=ztile[:]
        )
        nc.sync.dma_start(
            out=cv_dram[b][PAD + BQS : BTOT].rearrange("(p x) -> p x", p=128),
            in_=ztile[:],
        )

    # ---------------- coords -> indices / weights --------------------------
    # cx/cy for this core's queries: [128, 8]  (partition p = query-in-block,
    # free b = block)
    cxs = coordp.tile([128, NBLK], f32, name="cxs")
    cys = coordp.tile([128, NBLK], f32, name="cys")
    nc.scalar.dma_start(out=cxs[:], in_=crd[0, :].rearrange("(b p) -> p b", p=128))
    nc.scalar.dma_start(out=cys[:], in_=crd[1, :].rearrange("(b p) -> p b", p=128))

    # per-query element base offset of its pyramid inside its block's buffer
    pq_i = coordp.tile([128, 1], i32, name="pq_i")
    nc.gpsimd.iota(pq_i[:], pattern=[[1, 1]], base=0, channel_multiplier=1)
    bqf = coordp.tile([128, 1], f32, name="bqf")
    nc.vector.tensor_copy(out=bqf[:], in_=pq_i[:])
    nc.vector.tensor_scalar_mul(bqf[:], bqf[:], float(LVLSUM))

    idx_i = coordp.tile([128, NBLK, NUM_LEVELS], i32, name="idx_i")
    wx0e = coordp.tile([128, NBLK, NUM_LEVELS, KK], bf16, name="wx0e")
    wx1e = coordp.tile([128, NBLK, NUM_LEVELS, KK], bf16, name="wx1e")
    wy0e = coordp.tile([128, NBLK, NUM_LEVELS, KK], bf16, name="wy0e")
    wy1e = coordp.tile([128, NBLK, NUM_LEVELS, KK], bf16, name="wy1e")

    # c ramp: -4..5 (patch-col -> absolute offset from floor(coord))
    cramp_i = const_pool.tile([128, PS], i32, name="cramp_i")
    nc.gpsimd.iota(cramp_i[:], pattern=[[1, PS]], base=-RADIUS, channel_multiplier=0)
    crampf = const_pool.tile([128, PS], f32, name="crampf")
    nc.vector.tensor_copy(out=crampf[:], in_=cramp_i[:])

    def floor_frac(src, lvl, nm):
        """src [128,8] f32 coords (>=0) -> (floor f32, frac f32) at level lvl.

        The f32->i32 cast rounds-to-nearest on HW (truncates in CoreSim), so
        correct round-ups back down with an is_gt adjustment."""
        inv = 1.0 / (1 << lvl)
        xs = small.tile([128, NBLK], f32, name=f"xs_{nm}", tag="xs")
        nc.vector.tensor_scalar_mul(xs[:], src[:], inv)
        ii = small.tile([128, NBLK], i32, name=f"ii_{nm}", tag="ii")
        nc.vector.tensor_copy(out=ii[:], in_=xs[:])
        ff = small.tile([128, NBLK], f32, name=f"ff_{nm}", tag="ff")
        nc.vector.tensor_copy(out=ff[:], in_=ii[:])
        adj = small.tile([128, NBLK], f32, name=f"adj_{nm}", tag="adj")
        nc.vector.tensor_tensor(adj[:], ff[:], xs[:], op=Alu.is_gt)
        nc.vector.tensor_tensor(ff[:], ff[:], adj[:], op=Alu.subtract)  # floor
        fr = small.tile([128, NBLK], f32, name=f"fr_{nm}", tag="fr")
        nc.vector.tensor_tensor(fr[:], xs[:], ff[:], op=Alu.subtract)   # frac
        return ff, fr

    for l in range(NUM_LEVELS):
        Wl, Hl = LVL_W[l], LVL_H[l]
        ixf, fx = floor_frac(cxs, l, f"x{l}")
        iyf, fy = floor_frac(cys, l, f"y{l}")

        # interp weights with the OOB zero-mask folded in
        for (name_t, frac, posf, lim) in (
            ((wx0e, wx1e), fx, ixf, Wl - 1),
            ((wy0e, wy1e), fy, iyf, Hl - 1),
        ):
            w0t, w1t = name_t
            # tap positions posf-4+c for c in 0..9
            pos = small.tile([128, NBLK, PS], f32, name=f"pos{l}", tag="pos")
            nc.vector.tensor_tensor(
                pos[:],
                posf[:].unsqueeze(2).to_broadcast([128, NBLK, PS]),
                crampf[:, 0:PS].unsqueeze(1).to_broadcast([128, NBLK, PS]),
                op=Alu.add,
            )
            ok = small.tile([128, NBLK, PS], f32, name=f"ok{l}", tag="ok")
            nc.vector.tensor_scalar(pos[:], pos[:], float(lim) / 2.0, 2.0,
                                    op0=Alu.subtract, op1=Alu.mult)
            # now pos = 2*p - lim; in-bounds <=> |pos| <= lim
            nc.scalar.activation(ok[:], pos[:], mybir.ActivationFunctionType.Abs)
            nc.vector.tensor_scalar(ok[:], ok[:], float(lim), None, op0=Alu.is_le)
            w0 = small.tile([128, NBLK], f32, name=f"w0_{l}", tag="w0")
            nc.vector.tensor_scalar(w0[:], frac[:], -1.0, 1.0,
                                    op0=Alu.mult, op1=Alu.add)  # 1 - frac
            nc.vector.tensor_tensor(
                w0t[:, :, l, :],
                w0[:].unsqueeze(2).to_broadcast([128, NBLK, KK]),
                ok[:, :, 0:KK],
                op=Alu.mult,
            )
            nc.vector.tensor_tensor(
                w1t[:, :, l, :],
                frac[:].unsqueeze(2).to_broadcast([128, NBLK, KK]),
                ok[:, :, 1:PS],
                op=Alu.mult,
            )

        # gather run start: PAD + p*LVLSUM + lvl_off + (iy-4)*Wl + (ix-4)
        t1 = small.tile([128, NBLK], f32, name=f"t1_{l}", tag="t1")
        nc.vector.tensor_scalar_mul(t1[:], iyf[:], float(Wl))
        nc.vector.tensor_tensor(t1[:], t1[:], ixf[:], op=Alu.add)
        nc.vector.tensor_tensor(
            t1[:], t1[:], bqf[:].to_broadcast([128, NBLK]), op=Alu.add
        )
        nc.vector.tensor_scalar_add(
            t1[:], t1[:], float(PAD + LVL_OFF[l] - RADIUS * Wl - RADIUS)
        )
        nc.vector.tensor_copy(out=idx_i[:, :, l], in_=t1[:])  # exact ints

    # ---------------- fmap2 load + pyramid pooling -------------------------
    # f2 as two K-halves [128 chan, 8192 pix] bf16; pooled levels keep raw
    # SUMS, the 1/16 * 0.25^l scale is folded into the PSUM drain.
    f2_lv = []
    halves = []
    for k in range(2):
        f2h = f2_pool.tile([128, LVL_N[0]], bf16, name=f"f2h{k}")
        nc.sync.dma_start(out=f2h[:], in_=f2[k * 128 : (k + 1) * 128, :])
        halves.append(f2h)
    f2_lv.append(halves)
    for l in range(1, NUM_LEVELS):
        Wl, Hl = LVL_W[l], LVL_H[l]
        pw, ph = LVL_W[l - 1], LVL_H[l - 1]
        halves = []
        for k in range(2):
            prev = f2_lv[l - 1][k][:].rearrange(
                "p (h w two) -> p h w two", h=ph, w=pw // 2, two=2
            )
            s1 = small.tile(
                [128, ph, pw // 2], bf16, name=f"s1_{l}_{k}", tag="poolt", bufs=1
            )
            nc.vector.tensor_tensor(
                s1[:], prev[:, :, :, 0], prev[:, :, :, 1], op=Alu.add
            )
            s1v = s1[:].rearrange("p (h2 two) w -> p h2 two w", h2=Hl, two=2)
            cur = f2_pool.tile([128, Hl * Wl], bf16, name=f"f2l{l}_{k}")
            curv = cur[:].rearrange("p (h w) -> p h w", h=Hl, w=Wl)
            nc.vector.tensor_tensor(
                curv[:], s1v[:, :, 0, :], s1v[:, :, 1, :], op=Alu.add
            )
            halves.append(cur)
        f2_lv.append(halves)

    # ---------------- fmap1 load (once) ------------------------------------
    f1h = []
    for k in range(2):
        t = f1_pool.tile([128, QPC], bf16, name=f"f1h{k}")
        nc.sync.dma_start(out=t[:], in_=f1c[k * 128 : (k + 1) * 128, :])
        f1h.append(t)

    # ---------------- patch tiles (gather destinations) --------------------
    patch = [
        patchp.tile([128, NBLK, ROWL[l]], bf16, name=f"patch{l}")
        for l in range(NUM_LEVELS)
    ]

    # ---------------- per-block GEMM -> cv write -> gather ------------------
    drain_parity = 0
    for b in range(NBLK):
        cv_sb = cvp.tile([128, LVLSUM], bf16, name="cv_sb", tag="cv_sb")
        for l in range(NUM_LEVELS):
            scale_l = (1.0 / 16.0) * (0.25 ** l)
            Nl = LVL_N[l]
            for n0 in range(0, Nl, MM_N):
                n1 = min(Nl, n0 + MM_N)
                pt = psum.tile([128, n1 - n0], f32, name="pt", tag="pt")
                nc.tensor.matmul(
                    pt[:],
                    f1h[0][:, b * 128 : (b + 1) * 128],
                    f2_lv[l][0][:, n0:n1],
                    start=True,
                    stop=False,
                )
                nc.tensor.matmul(
                    pt[:],
                    f1h[1][:, b * 128 : (b + 1) * 128],
                    f2_lv[l][1][:, n0:n1],
                    start=False,
                    stop=True,
                )
                dst = cv_sb[:, LVL_OFF[l] + n0 : LVL_OFF[l] + n1]
                if drain_parity % 2 == 0:
                    nc.vector.tensor_scalar_mul(dst[:], pt[:], scale_l)
                else:
                    nc.scalar.mul(dst[:], pt[:], scale_l)
                drain_parity += 1
        # one DMA for the whole block's cv (rows 21.76KB contiguous)
        nc.sync.dma_start(
            out=cv_dram[b][PAD : PAD + BQS].rearrange("(q s) -> q s", s=LVLSUM),
            in_=cv_sb[:],
        )
        # one indirect gather per level: contiguous run covering the patch
        cv2d = cv_dram[b][:].rearrange("(a x) -> a x", a=1024)
        for l in range(NUM_LEVELS):
            nc.gpsimd.indirect_dma_start(
                out=patch[l][:, b, 0 : RUN[l]],
                out_offset=None,
                in_=cv2d,
                in_offset=bass.IndirectOffsetOnAxis(
                    ap=idx_i[:, b, l].unsqueeze(1), axis=1
                ),
            )

    if dbg is not None:
        nc.sync.dma_start(
            out=dbg["idx"][:], in_=idx_i[:].rearrange("p b l -> p (b l)")
        )
        nc.sync.dma_start(
            out=dbg["patch0"][:], in_=patch[0][:].rearrange("p b r -> p (b r)")
        )
        nc.sync.dma_start(
            out=dbg["patch3"][:], in_=patch[3][:].rearrange("p b r -> p (b r)")
        )
        nc.sync.dma_start(
            out=dbg["wx0"][:], in_=wx0e[:].rearrange("p b l k -> p (b l k)")
        )
        nc.sync.dma_start(
            out=dbg["cv0"][:],
            in_=cv_dram[0][0 : PAD + 2 * LVLSUM].rearrange("(p x) -> p x", p=128),
        )

    # ---------------- separable bilinear (batched over blocks) -------------
    outq = patchp.tile([128, NBLK, NUM_LEVELS, KK, KK], bf16, name="outq")
    for l in range(NUM_LEVELS):
        Wl = LVL_W[l]
        Pv = patch[l][:].rearrange("p b (r c) -> p b r c", r=PS, c=Wl)
        bshape_x = [128, NBLK, PS, KK]
        tx = txp.tile([128, NBLK, PS, KK], bf16, name=f"tx{l}", tag="tx")
        tx2 = txp.tile([128, NBLK, PS, KK], bf16, name=f"tx2{l}", tag="tx2")
        nc.vector.tensor_tensor(
            tx[:], Pv[:, :, :, 0:KK],
            wx0e[:, :, l, :].unsqueeze(2).to_broadcast(bshape_x), op=Alu.mult,
        )
        nc.vector.tensor_tensor(
            tx2[:], Pv[:, :, :, 1:PS],
            wx1e[:, :, l, :].unsqueeze(2).to_broadcast(bshape_x), op=Alu.mult,
        )
        nc.vector.tensor_tensor(tx[:], tx[:], tx2[:], op=Alu.add)

        bshape_y = [128, NBLK, KK, KK]
        oq2 = txp.tile([128, NBLK, KK, KK], bf16, name=f"oq2{l}", tag="oq2")
        nc.vector.tensor_tensor(
            oq2[:], tx[:, :, 0:KK, :],
            wy0e[:, :, l, :].unsqueeze(3).to_broadcast(bshape_y), op=Alu.mult,
        )
        nc.vector.tensor_tensor(
            outq[:, :, l], tx[:, :, 1:PS, :],
            wy1e[:, :, l, :].unsqueeze(3).to_broadcast(bshape_y), op=Alu.mult,
        )
        nc.vector.tensor_tensor(
            outq[:, :, l], outq[:, :, l], oq2[:], op=Alu.add
        )

    if dbg is not None:
        nc.sync.dma_start(
            out=dbg["outq"][:], in_=outq[:].rearrange("p b l dy dx -> p (b l dy dx)")
        )

    # ---------------- transpose to channel-major + store -------------------
    ident = const_pool.tile([128, 128], bf16, name="ident")
    make_identity(nc, ident[:])
    outq_v = outq[:].rearrange("p b l dy dx -> p b (l dy dx)")
    CHUNKS = [(0, 128), (128, 128), (256, NCH - 256)]
    for k, (c0, nk) in enumerate(CHUNKS):
        outTk = outp.tile([128, NBLK, 128], f32, name="outTk", tag="outTk")
        for b in range(NBLK):
            ptt = psum_t.tile([128, 128], bf16, name="ptt", tag="ptt")
            nc.tensor.transpose(
                out=ptt[:nk, :], in_=outq_v[:, b, c0 : c0 + nk], identity=ident[:]
            )
            if b % 2 == 0:
                nc.vector.tensor_copy(out=outTk[0:nk, b, :], in_=ptt[:nk, :])
            else:
                nc.scalar.copy(out=outTk[0:nk, b, :], in_=ptt[:nk, :])
        nc.sync.dma_start(
            out=out_ext[c0 : c0 + nk, :, :], in_=outTk[0:nk, :, :]
        )


def build_program(debug=False):
    """Build (once) the single-core SPMD bass program."""
    key = ("nc", debug)
    if key in _CACHE:
        return _CACHE[key]
    import concourse.tile as tile
    import concourse.mybir as mybir
    from concourse import bacc

    f32 = mybir.dt.float32
    bf16 = mybir.dt.bfloat16
    i32 = mybir.dt.int32
    nc = bacc.Bacc(
        "TRN2",
        target_bir_lowering=False,
        debug=False,
        enable_asserts=True,
        num_devices=NCORES,
    )
    f1c = nc.dram_tensor("f1c", [D, QPC], bf16, kind="ExternalInput").ap()
    f2 = nc.dram_tensor("f2", [D, H * W], bf16, kind="ExternalInput").ap()
    crd = nc.dram_tensor("crd", [2, QPC], f32, kind="ExternalInput").ap()
    out = nc.dram_tensor("out", [NCH, H // NCORES, W], f32, kind="ExternalOutput").ap()
    dbg = None
    if debug:
        dbg = {
            "idx": nc.dram_tensor(
                "dbg_idx", [128, NBLK * NUM_LEVELS], i32, kind="ExternalOutput"
            ).ap(),
            "patch0": nc.dram_tensor(
                "dbg_patch0", [128, NBLK * ROWL[0]], bf16, kind="ExternalOutput"
            ).ap(),
            "patch3": nc.dram_tensor(
                "dbg_patch3", [128, NBLK * ROWL[3]], bf16, kind="ExternalOutput"
            ).ap(),
            "wx0": nc.dram_tensor(
                "dbg_wx0", [128, NBLK * NUM_LEVELS * KK], bf16, kind="ExternalOutput"
            ).ap(),
            "cv0": nc.dram_tensor(
                "dbg_cv0", [128, (PAD + 2 * LVLSUM) // 128], bf16,
                kind="ExternalOutput",
            ).ap(),
            "outq": nc.dram_tensor(
                "dbg_outq", [128, NBLK * NCH], bf16, kind="ExternalOutput"
            ).ap(),
        }

    from contextlib import ExitStack

    with tile.TileContext(nc) as tc, ExitStack() as ctx:
        _emit(ctx, tc, out, f1c, f2, crd, dbg=dbg)
    nc.compile()
    _CACHE[key] = nc
    return nc


def make_in_maps(fmap1, fmap2, coords):
    import ml_dtypes

    bf = ml_dtypes.bfloat16
    f1 = np.ascontiguousarray(
        np.asarray(fmap1, dtype=np.float32).reshape(D, H * W)
    ).astype(bf)
    f2 = np.ascontiguousarray(
        np.asarray(fmap2, dtype=np.float32).reshape(D, H * W)
    ).astype(bf)
    crd = np.asarray(coords, dtype=np.float32).reshape(2, H * W)
    in_maps = []
    for c in range(NCORES):
        sl = slice(c * QPC, (c + 1) * QPC)
        in_maps.append(
            {
                "f1c": np.ascontiguousarray(f1[:, sl]),
                "f2": f2,
                "crd": np.ascontiguousarray(crd[:, sl]),
            }
        )
    return in_maps


def kernel(fmap1, fmap2, coords):
    from concourse.bass_utils import run_bass_kernel_spmd

    nc = build_program()
    in_maps = make_in_maps(fmap1, fmap2, coords)
    res = run_bass_kernel_spmd(nc, in_maps, list(range(NCORES)))
    parts = [res.results[c]["out"] for c in range(NCORES)]  # [324, 8, 128] each
    full = np.concatenate(parts, axis=1)  # [324, 64, 128]
    return full[None].astype(np.float32)
